# revision 44
# baseline (speedup 1.0000x reference)
"""Llama MHA (B=2, S=2048, D=2048, H=16, causal, RoPE) on 8 trn2 cores.

Sharding: data-parallel over batch (2 groups of 4 cores) x tensor-parallel
over heads (4 heads per core). Single-core program per core; host splits
inputs and sums the 4 out-projection partials per batch.

v2 design notes (vs the phase-sequential baseline):
- Phases interleaved per 512-seq chunk: proj(sc) -> attn(sc) -> outproj(sc),
  with proj(sc+1)/outproj(sc-1) matmuls emitted as fillers inside the
  attention loop so the PE never starves while exp runs.
- exp is done on PAIRS of score blocks ([128,1024] across 2 PSUM banks),
  halving the scalar engine's fixed per-instruction overhead.
- softmax denominator: DVE bf16 accumulation of exp tiles + one ones-matmul
  per (head, chunk) -- replaces 160 PE ones-matmuls with 16.
- causal trim: PV matmuls / exp / acc adds only touch the un-masked column
  range of diagonal blocks.
- all PSUM->SBUF drains (q/k pre-RoPE, v, out-proj) run on GpSimd; scalar
  does only exp; RoPE runs on DVE in bf16 (2x mode).
- host pre-permutes every DRAM tensor into the exact SBUF tile layout so
  DMAs are contiguous; output is bf16.
"""

import numpy as np
import ml_dtypes

import concourse.bass as bass
import concourse.mybir as mybir
import concourse.tile as tile
from concourse import bacc
from concourse.bass_utils import run_bass_kernel_spmd

B, S, D, H = 2, 2048, 2048, 16
DH = D // H            # 128 head dim
HPC = 4                # heads per core
N_CORES = 8
FH = HPC * DH          # 512 features per core
P = 128
KT = D // P            # 16 k-tiles over D
SC = S // 512          # 4 seq chunks of 512
ST = S // P            # 16 seq blocks of 128
THETA = 10000.0
SCALE = 1.0 / np.sqrt(DH)

DT = mybir.dt.bfloat16
F32 = mybir.dt.float32
NPDT = ml_dtypes.bfloat16

_prog_cache = {}


class FillerQueue:
    """Deferred PE work (closures) popped between attention pairs."""

    def __init__(self):
        self.q = []

    def push(self, fn, cost=1):
        self.q.append((fn, cost))

    def pop_cost(self, budget):
        while budget > 0 and self.q:
            fn, cost = self.q.pop(0)
            fn()
            budget -= cost

    def drain(self):
        while self.q:
            fn, _ = self.q.pop(0)
            fn()


def _build():
    if "nc" in _prog_cache:
        return _prog_cache["nc"]
    nc = bacc.Bacc(None, target_bir_lowering=False, debug=False)

    xd = nc.dram_tensor("xd", [SC, P, KT, 512], DT, kind="ExternalInput")
    wqd = nc.dram_tensor("wqd", [HPC, P, KT, DH], DT, kind="ExternalInput")
    wkd = nc.dram_tensor("wkd", [HPC, P, KT, DH], DT, kind="ExternalInput")
    wvd = nc.dram_tensor("wvd", [P, KT, FH], DT, kind="ExternalInput")
    wod = nc.dram_tensor("wod", [P, HPC, D], DT, kind="ExternalInput")
    ccd = nc.dram_tensor("ccd", [P, S], DT, kind="ExternalInput")
    ssd = nc.dram_tensor("ssd", [P, S], DT, kind="ExternalInput")
    trid = nc.dram_tensor("trid", [P, P], DT, kind="ExternalInput")
    resT = nc.dram_tensor("resT", [SC, KT // 4, P, 4, 512], DT,
                          kind="ExternalOutput")

    with tile.TileContext(nc) as tc:
        with (
            tc.tile_pool(name="persist", bufs=1) as pp,
            tc.tile_pool(name="qtc", bufs=2) as qp,
            tc.tile_pool(name="attnc", bufs=2) as ap,
            tc.tile_pool(name="pqb", bufs=8) as bp,
            tc.tile_pool(name="rope", bufs=2) as rp,
            tc.tile_pool(name="pt", bufs=3) as tp,
            tc.tile_pool(name="accp", bufs=2) as cp,
            tc.tile_pool(name="bcp", bufs=2) as vp,
            tc.tile_pool(name="rtp", bufs=3) as op_,
            tc.tile_pool(name="xcp", bufs=2) as xp,
            tc.tile_pool(name="psP", bufs=2, space="PSUM") as psP,
            tc.tile_pool(name="psA", bufs=2, space="PSUM") as psA,
            tc.tile_pool(name="psB", bufs=2, space="PSUM") as psB,
        ):
            kT = pp.tile([P, HPC, S], DT)      # rope'd k^T, all chunks
            vn = pp.tile([P, ST, FH], DT)      # v natural [seq-block, feat]
            cc_t = pp.tile([P, S], DT)
            ss_t = pp.tile([P, S], DT)
            tri = pp.tile([P, P], DT)          # tri[k,u] = (k <= u)
            ones_mat = pp.tile([P, P], DT)
            wq_t = pp.tile([P, HPC, KT, DH], DT)
            wk_t = pp.tile([P, HPC, KT, DH], DT)
            wv_t = pp.tile([P, KT, FH], DT)
            wo_t = pp.tile([P, HPC, D], DT)

            nc.vector.memset(ones_mat, 1.0)

            # ---------------- preamble DMAs --------------------------------
            # Critical pieces for proj(0)'s first chains go on the scalar
            # HWDGE queue (starts transferring ~1.5us in); the rest go on the
            # sync queue, whose ~8us init backlog naturally sequences them
            # after the criticals without stealing DMA bandwidth.
            xc = [None] * SC
            xc[0] = xp.tile([P, KT, 512], DT, tag="xc", name="xc0")
            # wq stream (the kgroup pacer) + the first x piece on sync, which
            # starts transferring earliest; rest of x + bulk on scalar.
            nc.sync.dma_start(out=wq_t[:, 0, 0:2, :], in_=wqd[0, :, 0:2, :])
            nc.sync.dma_start(out=xc[0][:, 0:2, :], in_=xd[0, :, 0:2, :])
            nc.sync.dma_start(out=wq_t[:, 0, 2:16, :], in_=wqd[0, :, 2:16, :])
            nc.sync.dma_start(out=wq_t[:, 1, :, :], in_=wqd[1, :, :, :])
            nc.sync.dma_start(out=wq_t[:, 2, :, :], in_=wqd[2, :, :, :])
            nc.sync.dma_start(out=wq_t[:, 3, :, :], in_=wqd[3, :, :, :])
            nc.sync.dma_start(out=cc_t[:, 0:512], in_=ccd[:, 0:512])
            nc.sync.dma_start(out=ss_t[:, 0:512], in_=ssd[:, 0:512])
            nc.scalar.dma_start(out=xc[0][:, 2:4, :], in_=xd[0, :, 2:4, :])
            nc.scalar.dma_start(out=xc[0][:, 4:8, :], in_=xd[0, :, 4:8, :])
            nc.scalar.dma_start(out=xc[0][:, 8:16, :], in_=xd[0, :, 8:16, :])
            for h in range(HPC):
                nc.scalar.dma_start(out=wk_t[:, h, :, :], in_=wkd[h, :, :, :])
            nc.scalar.dma_start(out=wv_t, in_=wvd[:, :, :])
            nc.scalar.dma_start(out=cc_t[:, 512:], in_=ccd[:, 512:])
            nc.scalar.dma_start(out=ss_t[:, 512:], in_=ssd[:, 512:])
            nc.scalar.dma_start(out=tri, in_=trid[:, :])
            xc[1] = xp.tile([P, KT, 512], DT, tag="xc", name="xc1")
            nc.scalar.dma_start(out=xc[1], in_=xd[1, :, :, :])
            nc.scalar.dma_start(out=wo_t, in_=wod[:, :, :])

            qTc = [None] * SC    # current-chunk rope'd q
            attnc = [None] * SC  # current-chunk attention output

            def emit_rope_ops(pqb, dst, csl):
                ta = rp.tile([P, 512], DT, tag="ta")
                tb = rp.tile([P, 512], DT, tag="tb")
                nc.vector.tensor_mul(ta, pqb, cc_t[:, csl])
                nc.vector.tensor_mul(
                    tb[0:64, :], pqb[64:128, :], ss_t[64:128, csl])
                nc.vector.tensor_mul(
                    tb[64:128, :], pqb[0:64, :], ss_t[0:64, csl])
                nc.vector.tensor_add(dst, ta, tb)

            def emit_proj0():
                """proj(0) with q-chains interleaved by k-group so the PE
                consumes each xc piece as it lands (supply-paced start)."""
                csl = slice(0, 512)
                qs = {}
                pbq = psB.tile([P, 1024], F32, tag="pb", name="pbq0")
                for h in range(HPC):
                    if h < 2:
                        qs[h] = psP.tile([P, 512], F32, tag="ps",
                                         name=f"ps_q{h}_0")
                    else:
                        qs[h] = pbq[:, (h - 2) * 512:(h - 1) * 512]
                # (q0,q1) k-grouped first -- they only need wq0/wq1 early;
                # by the time (q2,q3) start, their weights have landed.
                for hpair in ((0, 1), (2, 3)):
                    for k0, k1 in ((0, 2), (2, 4), (4, 8), (8, 16)):
                        for h in hpair:
                            for k in range(k0, k1):
                                nc.tensor.matmul(
                                    qs[h], wq_t[:, h, k, :], xc[0][:, k, :],
                                    start=(k == 0), stop=(k == KT - 1),
                                    skip_group_check=True)
                qTc[0] = qp.tile([P, HPC, 512], DT, tag="qt", name="qt0")
                for h in range(HPC):
                    pqb = bp.tile([P, 512], DT, tag="pqb", name=f"pqb_q{h}_0")
                    nc.vector.tensor_copy(pqb, qs[h])
                    emit_rope_ops(pqb, qTc[0][:, h, :], csl)
                for h in range(HPC):
                    pk = psP.tile([P, 512], F32, tag="ps", name=f"ps_k{h}_0")
                    for k in range(KT):
                        nc.tensor.matmul(
                            pk, wk_t[:, h, k, :], xc[0][:, k, :],
                            start=(k == 0), stop=(k == KT - 1))
                    pqb = bp.tile([P, 512], DT, tag="pqb", name=f"pqb_k{h}_0")
                    nc.vector.tensor_copy(pqb, pk)
                    emit_rope_ops(pqb, kT[:, h, csl], csl)
                for st4 in range(4):
                    pv = psP.tile([P, 512], F32, tag="ps", name=f"ps_v{st4}_0")
                    for k in range(KT):
                        emit_vchain_matmul(pv, st4, k, xc[0])
                    nc.vector.tensor_copy(vn[:, st4, :], pv)

            def emit_chain_matmul(ps, wt, h, k, xcc):
                nc.tensor.matmul(
                    ps, wt[:, h, k, :], xcc[:, k, :],
                    start=(k == 0), stop=(k == KT - 1),
                )

            def emit_vchain_matmul(ps, st4, k, xcc):
                nc.tensor.matmul(
                    ps, xcc[:, k, st4 * P:(st4 + 1) * P], wv_t[:, k, :],
                    start=(k == 0), stop=(k == KT - 1),
                )

            def make_proj_closures(sc, fq):
                """Queue proj(sc)'s chain matmuls + drains as fillers."""
                csl = slice(sc * 512, (sc + 1) * 512)
                state = {}

                def start_chain(key):
                    ps = psP.tile([P, 512], F32, tag="ps", name=f"ps_{key}_{sc}")
                    state[key] = ps
                    return ps

                for wt, kind in ((wq_t, "q"), (wk_t, "k")):
                    for h in range(HPC):
                        key = f"{kind}{h}"
                        for k in range(KT):
                            def mm(k=k, h=h, wt=wt, key=key):
                                ps = state[key] if k else start_chain(key)
                                emit_chain_matmul(ps, wt, h, k, xc[sc])
                            fq.push(mm, 1)

                        def drain(key=key, kind=kind, h=h):
                            pqb = bp.tile([P, 512], DT, tag="pqb",
                                          name=f"pqb_{key}_{sc}")
                            nc.vector.tensor_copy(pqb, state[key])
                            state[key + "_b"] = pqb

                        def rope(key=key, kind=kind, h=h):
                            if qTc[sc] is None:
                                qTc[sc] = qp.tile([P, HPC, 512], DT, tag="qt",
                                                  name=f"qt{sc}")
                            pqb = state[key + "_b"]
                            ta = rp.tile([P, 512], DT, tag="ta")
                            tb = rp.tile([P, 512], DT, tag="tb")
                            nc.vector.tensor_mul(ta, pqb, cc_t[:, csl])
                            nc.vector.tensor_mul(
                                tb[0:64, :], pqb[64:128, :], ss_t[64:128, csl])
                            nc.vector.tensor_mul(
                                tb[64:128, :], pqb[0:64, :], ss_t[0:64, csl])
                            dst = (qTc[sc][:, h, :] if kind == "q"
                                   else kT[:, h, csl])
                            nc.vector.tensor_add(dst, ta, tb)
                        fq.push(drain, 0)
                        fq.push(rope, 0)
                for st4 in range(4):
                    key = f"v{st4}"
                    for k in range(KT):
                        def mm(k=k, st4=st4, key=key):
                            ps = state[key] if k else start_chain(key)
                            emit_vchain_matmul(ps, st4, k, xc[sc])
                        fq.push(mm, 1)

                    def drain(st4=st4, key=key):
                        nc.vector.tensor_copy(vn[:, sc * 4 + st4, :],
                                              state[key])
                    fq.push(drain, 0)

            def emit_attn(sc, fq):
                """Attention for q-chunk sc; pops fillers between pairs."""
                nkb = 4 * (sc + 1)
                attnc[sc] = ap.tile([P, HPC, 512], DT, tag="at", name=f"at{sc}")
                for h in range(HPC):
                    fsl = slice(h * DH, (h + 1) * DH)
                    po = psA.tile([P, 512], F32, tag="po", name=f"po{sc}{h}")
                    acc = cp.tile([P, 512], DT, tag="acc", name=f"acc{sc}{h}")
                    for pi in range(nkb // 2):
                        kb0 = 2 * pi
                        # j-index of each block on the block-diagonal (<0: off)
                        j0 = kb0 - 4 * sc
                        j1 = kb0 + 1 - 4 * sc
                        o0 = max(0, 128 * j0)
                        o1 = max(0, 128 * j1)
                        pb = psB.tile([P, 1024], F32, tag="pb",
                                      name=f"pb{sc}{h}{pi}")
                        nc.tensor.matmul(
                            pb[:, o0:512], kT[:, h, kb0 * P:(kb0 + 1) * P],
                            qTc[sc][:, h, o0:512], start=True, stop=True,
                        )
                        nc.tensor.matmul(
                            pb[:, 512 + o1:1024],
                            kT[:, h, (kb0 + 1) * P:(kb0 + 2) * P],
                            qTc[sc][:, h, o1:512], start=True, stop=True,
                        )
                        fq.pop_cost(2)
                        pt2 = tp.tile([P, 1024], DT, tag="pt",
                                      name=f"pt{sc}{h}{pi}")
                        if o1 > 0:
                            # diag pair: two exp calls skip the unused
                            # PSUM hole [512:512+o1]
                            nc.scalar.activation(
                                pt2[:, o0:512], pb[:, o0:512],
                                mybir.ActivationFunctionType.Exp,
                                scale=float(SCALE),
                            )
                            nc.scalar.activation(
                                pt2[:, 512 + o1:], pb[:, 512 + o1:],
                                mybir.ActivationFunctionType.Exp,
                                scale=float(SCALE),
                            )
                        else:
                            nc.scalar.activation(
                                pt2[:, o0:], pb[:, o0:],
                                mybir.ActivationFunctionType.Exp,
                                scale=float(SCALE),
                            )
                        # mask diagonal blocks (within-block triangle)
                        if j0 >= 0:
                            nc.vector.tensor_mul(
                                pt2[:, o0:o0 + 128], pt2[:, o0:o0 + 128], tri)
                        if j1 >= 0:
                            nc.vector.tensor_mul(
                                pt2[:, 512 + o1:512 + o1 + 128],
                                pt2[:, 512 + o1:512 + o1 + 128], tri)
                        # PV matmuls + denominator accumulation
                        for kb, off, o in ((kb0, 0, o0), (kb0 + 1, 512, o1)):
                            nc.tensor.matmul(
                                po[:, o:512], vn[:, kb, fsl],
                                pt2[:, off + o:off + 512],
                                start=(kb == 0), stop=(kb == nkb - 1),
                                skip_group_check=True,
                            )
                            if kb == 0:
                                nc.vector.tensor_copy(acc, pt2[:, 0:512])
                            else:
                                nc.vector.tensor_add(
                                    acc[:, o:], acc[:, o:],
                                    pt2[:, off + o:off + 512])
                    pd = psA.tile([P, 512], F32, tag="po", name=f"pd{sc}{h}")
                    nc.tensor.matmul(pd, ones_mat, acc, start=True, stop=True)
                    bc = vp.tile([P, 512], F32, tag="bc", name=f"bc{sc}{h}")
                    nc.vector.reciprocal_approx_fast(out=bc, in_=pd)
                    nc.vector.tensor_mul(attnc[sc][:, h, :], po, bc)

            def make_outproj_closures(sc, fq, alt_pool=False):
                """Queue outproj(sc): per db: 4 matmuls + drain + DMA."""
                state = {}
                for db in range(KT):
                    key = f"o{db}"
                    for ft in range(HPC):
                        def mm(ft=ft, db=db, key=key):
                            if ft == 0:
                                use_a = alt_pool and db % 2 == 0
                                pool = psA if use_a else psP
                                tg = "po" if use_a else "ps"
                                state[key] = pool.tile(
                                    [P, 512], F32, tag=tg,
                                    name=f"pr{sc}{db}")
                            nc.tensor.matmul(
                                state[key],
                                wo_t[:, ft, db * P:(db + 1) * P],
                                attnc[sc][:, ft, :],
                                start=(ft == 0), stop=(ft == HPC - 1),
                            )
                        fq.push(mm, 1)

                    def drain(db=db, key=key, sc=sc):
                        g, j = db // 4, db % 4
                        if j == 0:
                            state[f"rt{g}"] = op_.tile(
                                [P, 4, 512], DT, tag="rt", name=f"rt{sc}{g}")
                        rt = state[f"rt{g}"]
                        if sc == 2 or (sc == SC - 1 and db % 2 == 1):
                            # outproj(2) runs as attn(3) fillers where DVE is
                            # near-saturated; last chunk alternates so the
                            # final drains run on two engines concurrently.
                            nc.scalar.activation(
                                rt[:, j, :], state[key],
                                mybir.ActivationFunctionType.Copy)
                        else:
                            nc.vector.tensor_copy(rt[:, j, :], state[key])
                        if j % 2 == 1:
                            nc.sync.dma_start(
                                out=resT[sc, g, :, j - 1:j + 1, :],
                                in_=rt[:, j - 1:j + 1, :])
                    fq.push(drain, 0)

            # ---------------- main interleaved program --------------------
            fq = FillerQueue()

            # proj(0) solid
            emit_proj0()

            for sc in range(SC):
                # stage next chunk's proj + previous chunk's outproj
                if sc + 1 < SC:
                    if sc + 2 < SC:
                        xc[sc + 2] = xp.tile([P, KT, 512], DT, tag="xc",
                                             name=f"xc{sc + 2}")
                        nc.sync.dma_start(out=xc[sc + 2],
                                          in_=xd[sc + 2, :, :, :])
                    make_proj_closures(sc + 1, fq)
                if sc >= 1:
                    make_outproj_closures(sc - 1, fq)

                emit_attn(sc, fq)
                fq.drain()

            make_outproj_closures(SC - 1, fq, alt_pool=True)
            fq.drain()

    nc.finalize()
    _prog_cache["nc"] = nc
    return nc


def _host_inputs(x, w_q, w_k, w_v, w_o):
    """Build the 8 per-core input maps (DRAM laid out as SBUF tile images)."""
    i = np.arange(DH)
    perm_head = np.concatenate([i[0::2], i[1::2]])  # de-interleave pairs

    t = np.arange(S, dtype=np.float64)
    inv_freq = 1.0 / (THETA ** (np.arange(0, DH, 2, dtype=np.float64) / DH))
    ang = np.outer(t, inv_freq)               # [S, 64]
    cosT = np.cos(ang).T
    sinT = np.sin(ang).T
    ccd = np.vstack([cosT, cosT]).astype(NPDT)    # [128, S]
    ssd = np.vstack([sinT, -sinT]).astype(NPDT)   # +sin bottom, -sin top

    kk = np.arange(P)[:, None]
    uu = np.arange(P)[None, :]
    trid = (kk <= uu).astype(NPDT)            # [128, 128]

    in_maps = []
    for core in range(N_CORES):
        b = core // 4
        h0 = (core % 4) * HPC
        cols = np.concatenate(
            [(h0 + h) * DH + perm_head for h in range(HPC)])
        vcols = np.arange(h0 * DH, (h0 + HPC) * DH)

        wq_c = w_q[:, cols].reshape(KT, P, HPC, DH).transpose(2, 1, 0, 3)
        wk_c = w_k[:, cols].reshape(KT, P, HPC, DH).transpose(2, 1, 0, 3)
        wv_c = w_v[:, vcols].reshape(KT, P, FH).transpose(1, 0, 2)
        wo_c = w_o[vcols, :].reshape(HPC, P, D).transpose(1, 0, 2)
        x_c = x[b].reshape(SC, 512, KT, P).transpose(0, 3, 2, 1)

        in_maps.append({
            "xd": np.ascontiguousarray(x_c).astype(NPDT),
            "wqd": np.ascontiguousarray(wq_c).astype(NPDT),
            "wkd": np.ascontiguousarray(wk_c).astype(NPDT),
            "wvd": np.ascontiguousarray(wv_c).astype(NPDT),
            "wod": np.ascontiguousarray(wo_c).astype(NPDT),
            "ccd": ccd,
            "ssd": ssd,
            "trid": trid,
        })
    return in_maps


def kernel(x, w_q, w_k, w_v, w_o, _trace=False, _results_out=None):
    x = np.asarray(x, dtype=np.float32)
    w_q = np.asarray(w_q, dtype=np.float32)
    w_k = np.asarray(w_k, dtype=np.float32)
    w_v = np.asarray(w_v, dtype=np.float32)
    w_o = np.asarray(w_o, dtype=np.float32)
    nc = _build()
    in_maps = _host_inputs(x, w_q, w_k, w_v, w_o)
    res = run_bass_kernel_spmd(
        nc, in_maps, core_ids=list(range(N_CORES)), trace=_trace)
    if _results_out is not None:
        _results_out.append(res)
    out = np.empty((B, S, D), np.float32)
    for b in range(B):
        acc = res.results[4 * b]["resT"].astype(np.float32)
        for g in range(1, 4):
            acc = acc + res.results[4 * b + g]["resT"].astype(np.float32)
        # resT [SC, KT//4, P, 4, 512] -> [S, D]; d = g*512 + j*128 + p
        out[b] = acc.transpose(0, 4, 1, 3, 2).reshape(S, D)
    return out
